# revision 7
# baseline (speedup 1.0000x reference)
"""BPaCo+ loss on 8 TRN2 NeuronCores.

Two-path exp pipeline (scalar + vector engines both exp'ing) with the PE
held at its boosted 2.4 GHz p-state for the whole run. The TRN2 PE DVFS
heuristic (reverse-engineered on HW): 128-contraction plain matmuls raise
the clock after ~12 back-to-back instructions, DoubleRow (65-partition)
matmuls only hold it, and any semaphore wait drops it. So the kernel front-
loads a junk plain-matmul warmup chain that ramps the clock while the input
DMAs are still in flight, and pads every would-be PE wait with plain junk
matmuls accumulating into a scratch PSUM bank.

Per core, queue columns split 2560 (std) + 1536 (flip):
 - std path (scalar engine): per anchor-iblock, fp8 DoubleRow matmuls (the
   lnr(j) weight rides contraction row 128) fill a [128, 2560] PSUM tile;
   one Exp activation per iblock produces the softmax partial via the
   activation accumulator.
 - flip path (vector engine): per 128-queue-col block, plain fp8 matmuls
   (queue on PSUM partitions) + Schraudolph exp on the DVE (bits of
   bf16(e^x) = int16(A*x + B), with A*lnr+B as the per-partition scalar2),
   then the PE reduces over queue partitions with a ones-column bf16 matmul
   accumulating into a [2, 512] PSUM region (row h = anchor half h).

PSUM: 5 banks std + 2 banks flip + 1 bank dummy/reduce-acc... dummy shares
none: 5 + 2 + 0.5+0.5 -> dummy chain accumulates into the reduce bank's
free half? No: dummy gets its own region inside the flip-sum bank is not
possible; the warmup/filler accumulate into the FACC bank before its first
real use is racy, so the dummy bank doubles as the 8th bank and the flip
sums accumulate into rows 64+ of the same bank region (disjoint partitions
share a bank safely via a [128, 512] tile: rows 0:2 = flip sums, rows
64:128 = dummy target).

Host computes the O(B*(B+C)) blocks exactly, as before. The flip sums carry
a small systematic Schraudolph bias divided out on host (BETA_CAL).
"""
import numpy as np
import ml_dtypes

from concourse import bass, bacc, mybir, tile
from concourse.bass_utils import run_bass_kernel_spmd

B, K, C, D = 1024, 32768, 100, 128
T, ALPHA = 0.07, 0.05
M = 8                       # cores
QSH = K // M                # 4096 queue cols per core
QS = 2560                   # std-path queue cols
QF = QSH - QS               # 1536 flip-path queue cols
JB = QF // 128              # 12 flip blocks
NH = JB * 2                 # 24 flip half-tiles [128 queue, 512 anchor]
IB = 8                      # anchor i-blocks of 128

NWARM = 12                  # plain junk matmuls to ramp the PE clock
NDUM = 2                    # junk matmuls appended per phase

BF16 = mybir.dt.bfloat16
F32 = mybir.dt.float32
FP8 = mybir.dt.float8e4
I16 = mybir.dt.int16
NP_FP8 = ml_dtypes.float8_e4m3
NP_BF16 = ml_dtypes.bfloat16

# Schraudolph constants for bf16 bit pattern: bits = A1*x + B1OFF
A1 = float(1 << 7) / np.log(2.0)
B1OFF = 127.0 * (1 << 7) + 0.5 - 8.0
# systematic multiplicative bias of the bf16 Schraudolph exp; flip sums are
# divided by (1 + BETA_CAL) on host
BETA_CAL = 0.0

_CACHE = {}


def _build_nc():
    nc = bacc.Bacc(None, target_bir_lowering=False)
    # anchors DR-packed (fTq + ones row) for the std path, [65, 2, 512] x2
    X0 = nc.declare_dram_parameter("X0", [65, 1024], FP8, isOutput=False)
    X1 = nc.declare_dram_parameter("X1", [65, 1024], FP8, isOutput=False)
    # anchors plain [128, 1024] (fTq, no ones row) for the flip path
    XP = nc.declare_dram_parameter("XP", [128, 1024], FP8, isOutput=False)
    # queue std part, DR-packed with lnr row, [65, 2, 2560]
    RQS0 = nc.declare_dram_parameter("RQS0", [65, 1024], FP8, isOutput=False)
    RQS1 = nc.declare_dram_parameter("RQS1", [65, 2048], FP8, isOutput=False)
    RQS2 = nc.declare_dram_parameter("RQS2", [65, 2048], FP8, isOutput=False)
    # queue flip part, plain [128, 1536] features only
    RQF0 = nc.declare_dram_parameter("RQF0", [128, 512], FP8, isOutput=False)
    RQF1 = nc.declare_dram_parameter("RQF1", [128, 1024], FP8, isOutput=False)
    # per flip-block lnr scalars, pre-affined: A1*lnr0p + B1OFF, [128, JB]
    LNR = nc.declare_dram_parameter("LNR", [128, JB], F32, isOutput=False)
    ACC = nc.declare_dram_parameter("ACC", [128, 16], F32, isOutput=True)
    FACC = nc.declare_dram_parameter("FACC", [2, 512], F32, isOutput=True)

    with tile.TileContext(nc) as tc:
        with (
            tc.tile_pool(name="sb", bufs=1) as sbp,
            tc.tile_pool(name="ps", bufs=1, space=bass.MemorySpace.PSUM) as pps,
        ):
            X0_sb = sbp.tile([65, 2, 512], FP8, tag="X0")
            X1_sb = sbp.tile([65, 2, 512], FP8, tag="X1")
            XP_sb = sbp.tile([128, 1024], FP8, tag="XP")
            RQS_sb = sbp.tile([65, 2, QS], FP8, tag="RQS")
            RQF_sb = sbp.tile([128, QF], FP8, tag="RQF")
            LNR_sb = sbp.tile([128, JB], F32, tag="LNR")
            O01_sb = sbp.tile([128, 4], BF16, tag="O01")
            JNK_sb = sbp.tile([128, 512], BF16, tag="JNK")

            nc.scalar.dma_start(X0_sb[:], X0[:])
            nc.scalar.dma_start(RQS_sb[:, :, 0:512], RQS0[:])
            nc.scalar.dma_start(XP_sb[:], XP[:])
            nc.sync.dma_start(X1_sb[:], X1[:])
            nc.sync.dma_start(RQS_sb[:, :, 512:1536], RQS1[:])
            nc.sync.dma_start(LNR_sb[:], LNR[:])
            nc.gpsimd.dma_start(RQF_sb[:, 0:512], RQF0[:])
            nc.gpsimd.dma_start(RQS_sb[:, :, 1536:2560], RQS2[:])
            nc.gpsimd.dma_start(RQF_sb[:, 512:1536], RQF1[:])

            # ones columns for the flip reduction: O01[:, 0:2] = [1, 0],
            # O01[:, 2:4] = [0, 1]; junk zeros for warmup/filler matmuls
            nc.gpsimd.memset(JNK_sb[:], 0.0)
            nc.gpsimd.memset(O01_sb[:], 0.0)
            nc.gpsimd.memset(O01_sb[:, 0:1], 1.0)
            nc.gpsimd.memset(O01_sb[:, 3:4], 1.0)

            ACC_sb = sbp.tile([128, 16], F32, tag="ACCsb")
            FACC_sb = sbp.tile([2, 512], F32, tag="FACCsb")
            Etrash = sbp.tile([128, QS], BF16, tag="Etrash")

            warm = sbp.tile([128, 1], F32, tag="warm")
            nc.gpsimd.memset(warm[:], 0.0)
            nc.scalar.activation(
                warm[:], warm[:], mybir.ActivationFunctionType.Exp)

            SP = pps.tile([128, QS], F32, tag="SP")           # 5 banks
            RACC = pps.tile([2, 512], F32, tag="RACC")        # 1 bank

            # junk matmuls accumulate exact zeros into RACC (both operands
            # memset to 0), so the clock-management filler shares the
            # flip-sum bank instead of costing one
            ndum = 0

            def dummy(n):
                nonlocal ndum
                for _ in range(n):
                    nc.tensor.matmul(
                        RACC[:], JNK_sb[:, 0:2], JNK_sb[:, 0:512],
                        start=(ndum == 0), stop=False,
                        skip_group_check=True,
                    )
                    ndum += 1

            def xblk(b):
                t = X0_sb if b < 4 else X1_sb
                return t[:, :, (b % 4) * 128:(b % 4) * 128 + 128]

            DR = mybir.MatmulPerfMode.DoubleRow
            flip_q = 0
            done_q = 0
            epool = []

            def emit_flip_main():
                nonlocal flip_q
                if flip_q >= NH:
                    return
                q = flip_q
                jb, h = q // 2, q % 2
                FPt = pps.tile([128, 512], F32, tag="FP", bufs=2)
                nc.tensor.matmul(
                    FPt[:],
                    RQF_sb[:, jb * 128:jb * 128 + 128],
                    XP_sb[:, h * 512:h * 512 + 512],
                    start=True, stop=True,
                )
                e16 = sbp.tile([128, 512], I16, tag="E", bufs=4)
                nc.vector.tensor_scalar(
                    e16[:], FPt[:], A1, LNR_sb[:, jb:jb + 1],
                    mybir.AluOpType.mult, mybir.AluOpType.add,
                )
                epool.append((q, h, e16))
                flip_q += 1

            def emit_flip_reduce():
                nonlocal done_q
                if done_q >= len(epool):
                    return
                q, h, e16 = epool[done_q]
                nc.tensor.matmul(
                    RACC[:],
                    O01_sb[:, 2 * h:2 * h + 2],
                    e16[:].bitcast(BF16),
                    start=False, stop=(q == NH - 1),
                    skip_group_check=True,
                )
                done_q += 1

            def std_chunk(p, c0):
                nc.tensor.matmul(
                    SP[:, c0:c0 + 512],
                    xblk(p),
                    RQS_sb[:, :, c0:c0 + 512],
                    start=True, stop=True, perf_mode=DR,
                )

            dummy(NWARM)
            for p in range(IB):
                # PE order tuned so every dependency is satisfied on
                # arrival: std chunks land right as the split exp of the
                # previous phase releases their banks; flip mains are spaced
                # >= the DVE pass latency apart
                lag = len(epool)
                std_chunk(p, 0)
                std_chunk(p, 512)
                std_chunk(p, 1024)
                emit_flip_main()
                if done_q < lag:
                    emit_flip_reduce()
                std_chunk(p, 1536)
                std_chunk(p, 2048)
                emit_flip_main()
                if done_q < lag:
                    emit_flip_reduce()
                emit_flip_main()
                if done_q < lag:
                    emit_flip_reduce()
                dummy(NDUM)
                # split exp so the next phase's leading matmul chunks can
                # reuse banks as soon as their half is consumed
                nc.scalar.activation(
                    Etrash[:, 0:1536], SP[:, 0:1536],
                    mybir.ActivationFunctionType.Exp,
                    accum_out=ACC_sb[:, 2 * p:2 * p + 1],
                )
                nc.scalar.activation(
                    Etrash[:, 1536:QS], SP[:, 1536:QS],
                    mybir.ActivationFunctionType.Exp,
                    accum_out=ACC_sb[:, 2 * p + 1:2 * p + 2],
                )
            while flip_q < NH:
                emit_flip_main()
            while done_q < NH:
                emit_flip_reduce()

            nc.vector.tensor_copy(FACC_sb[:], RACC[:])
            nc.sync.dma_start(ACC[:], ACC_sb[:])
            nc.sync.dma_start(FACC[:], FACC_sb[:])

    nc.compile()
    return nc


def _prep_inputs(features, labels):
    f = features.astype(np.float64)
    lab = labels.astype(np.int64)
    ccount = np.bincount(lab, minlength=C).astype(np.float64)

    lnr0 = -np.log(ccount)
    s2 = -np.median(lnr0[lab])
    lnr0p = lnr0[lab] + s2

    fq = f.astype(NP_FP8).astype(np.float32)
    fTq = (f[:B] / T).astype(NP_FP8).astype(np.float32)

    lx = np.zeros((130, B), np.float32)
    lx[:D] = fTq.T
    lx[D] = 1.0
    X = np.ascontiguousarray(lx.reshape(65, 2, B)).astype(NP_FP8)
    X0 = np.ascontiguousarray(X[:, :, 0:512]).reshape(65, -1)
    X1 = np.ascontiguousarray(X[:, :, 512:1024]).reshape(65, -1)
    XPm = fTq.T.astype(NP_FP8)                       # [128, 1024]

    in_maps = []
    for c in range(M):
        jQ = slice(B + c * QSH, B + (c + 1) * QSH)
        fqc = fq[jQ]                                  # [4096, 128]
        lnc = lnr0p[jQ]
        rqs = np.zeros((130, QS), np.float32)
        rqs[:D] = fqc[:QS].T
        rqs[D] = lnc[:QS]
        rqs = rqs.reshape(65, 2, QS).astype(NP_FP8)
        rqf = fqc[QS:].T.astype(NP_FP8)               # [128, 1536]
        lnr_blk = (A1 * lnc[QS:].reshape(JB, 128).T + B1OFF).astype(np.float32)
        im = {
            "X0": X0, "X1": X1, "XP": XPm,
            "RQS0": np.ascontiguousarray(rqs[:, :, 0:512]).reshape(65, -1),
            "RQS1": np.ascontiguousarray(rqs[:, :, 512:1536]).reshape(65, -1),
            "RQS2": np.ascontiguousarray(rqs[:, :, 1536:2560]).reshape(65, -1),
            "RQF0": np.ascontiguousarray(rqf[:, 0:512]),
            "RQF1": np.ascontiguousarray(rqf[:, 512:1536]),
            "LNR": lnr_blk,
        }
        in_maps.append(im)
    return in_maps, s2


def kernel(features, sup_logits, centers, labels, _debug=False, _trace=False):
    if "nc" not in _CACHE:
        _CACHE["nc"] = _build_nc()
    nc = _CACHE["nc"]
    in_maps, s2 = _prep_inputs(features, labels)
    res = run_bass_kernel_spmd(nc, in_maps, core_ids=list(range(M)), trace=_trace)
    _CACHE["last"] = res

    S2q = np.zeros(B, np.float64)
    for c in range(M):
        acc = res.results[c]["ACC"].astype(np.float64)     # [128, 9]
        facc = res.results[c]["FACC"].astype(np.float64)   # [2, 512]
        S2q += (acc[:, 0:2 * IB:2] + acc[:, 1:2 * IB:2]).T.reshape(B)
        S2q += facc.reshape(B) / (1.0 + BETA_CAL)
    S2q *= np.exp(-s2)

    # ---- host blocks (exact): batch-vs-batch, branch 1, sup logits ----
    f = features.astype(np.float64)
    f32b = features.astype(np.float32)
    sup = sup_logits.astype(np.float64)
    lab = labels.astype(np.int64)
    labB = lab[:B]
    ccount = np.bincount(lab, minlength=C).astype(np.float64)
    cntB = np.bincount(labB, minlength=C).astype(np.float64)
    cc1 = cntB + 1.0

    cols = np.concatenate([f32b[:B], centers.astype(np.float32)], axis=0)
    LG = (f32b[:B] @ cols.T) / np.float32(T)          # [B, B+C]
    ELG = np.exp(LG.astype(np.float64))
    ELG[np.arange(B), np.arange(B)] = 0.0             # diag masked in both branches

    match_bb = labB[:, None] == labB[None, :]
    W2 = 1.0 / (ccount[labB][None, :] - ALPHA * match_bb)
    S2h = (ELG[:, :B] * W2).sum(1)
    oh = labB[:, None] == np.arange(C)[None, :]
    S2sup = (np.exp(sup) / (ccount[None, :] - oh)).sum(1)
    S2 = S2q + S2h + S2sup

    lab1 = np.concatenate([labB, np.arange(C)])
    match1 = labB[:, None] == lab1[None, :]
    W1 = 1.0 / (cc1[lab1][None, :] - match1)  # diag already zeroed in ELG
    S1 = (ELG * W1).sum(1)

    g2 = np.zeros((C, D))
    np.add.at(g2, lab, f)
    g1 = np.zeros((C, D))
    np.add.at(g1, labB, f[:B])
    g1 += centers.astype(np.float64)
    A2 = np.einsum("id,id->i", f[:B], g2[labB]) / T - 1.0 / T
    A1h = np.einsum("id,id->i", f[:B], g1[labB]) / T - 1.0 / T

    msum = 1.0 + ALPHA * (ccount[labB] - 1.0)
    numer2 = sup[np.arange(B), labB] + ALPHA * A2
    loss2 = np.mean(np.log(S2) - numer2 / msum)
    loss1 = np.mean(np.log(S1) - A1h / cntB[labB])
    return np.array(loss1 + loss2, dtype=np.float32)
